# revision 15
# baseline (speedup 1.0000x reference)
"""AutoRegressiveLSTM Trainium2 kernel (8-core data-parallel).

Per core: batch shard of 16 (padded to 32 on the stationary side).

  - Output feedback folded into the recurrence:
        out_{t-1} = h_{t-1} @ W_out.T + b_out
        gates_t   = x_t m_t @ W_ihx.T + h_{t-1} @ W_eff.T + b_eff
        W_eff = W_hh + W_io @ W_out,  b_eff = b_ih + b_hh + W_io @ b_out
    (t=0 adds one extra matmul round with a host-computed correction
     stationary o0 = output_t - (hn @ W_out.T + b_out))
  - Matmuls in float32r (1 cycle/row at N>=256): stationary = h^T
    [128 x 32] batch columns, moving = W_eff.T in 512-wide chunks.
    Gates land in PSUM [32 x 4096] = 8 banks, one 512-chunk per bank,
    chunk order i,i,g~,g~,f,f,o,o so the LSTM cell math can start early.
  - Cell math: DVE adds bias in place in PSUM, ACT applies
    sigmoid/tanh in place (tanh g~ goes to SBUF), DVE combines.
  - h [32 x 1024] is transposed back to h^T via 8 PE transpose ops into
    PSUM bank0 (already consumed), then copied to SBUF for the next step
    and DMA'd to HBM for the final phase.
  - Final phase: y^T = W_out @ H^T as a batched matmul over all steps.
Host side does masking, sharding, packing, W_eff fold, final transpose.
"""

import numpy as np

import concourse.bass as bass
import concourse.mybir as mybir
import concourse.tile as tile
from concourse import bacc
from concourse.bass_utils import run_bass_kernel_spmd

F32 = mybir.dt.float32
F32R = mybir.dt.float32r
AF = mybir.ActivationFunctionType

B, T, I, H, O = 128, 1024, 128, 1024, 128
NCORES = 8
BS = B // NCORES          # real batch per core (16)
BP = 32                   # padded batch per core
G = 4 * H                 # 4096 gates
NK = H // 128             # 8 h k-tiles
NCH = 8                   # gate chunks of 512
CW = 512                  # chunk width
# chunk n holds reference-gate rows 1024*CHUNK_TYPE[n] + 512*CHUNK_HALF[n] +:512
# reference gate order: i(0), f(1), g(2), o(3); we order i,i,g,g,f,f,o,o
CHUNK_TYPE = [0, 0, 2, 2, 1, 1, 3, 3]
CHUNK_HALF = [0, 1, 0, 1, 0, 1, 0, 1]


def chunk_rows(n):
    r0 = 1024 * CHUNK_TYPE[n] + 512 * CHUNK_HALF[n]
    return r0, r0 + CW


def build_program(t_steps=T + 1, unroll=8, final_tblk=32):
    """t_steps includes t=0; the loop runs (t_steps-1) steps (divisible by
    unroll)."""
    assert (t_steps - 1) % unroll == 0
    n_iters = (t_steps - 1) // unroll

    nc = bacc.Bacc(
        "TRN2", target_bir_lowering=False, debug=False, num_devices=NCORES
    )

    xT = nc.dram_tensor("xT", [128, t_steps * BP], F32R, kind="ExternalInput")
    Wstr = nc.dram_tensor("Wstr", [128, NK * NCH * CW], F32R, kind="ExternalInput")
    Wx = nc.dram_tensor("Wx", [128, NCH * CW], F32R, kind="ExternalInput")
    Wio = nc.dram_tensor("Wio", [128, NCH * CW], F32R, kind="ExternalInput")
    WoT = nc.dram_tensor("WoT", [128, NK * O], F32R, kind="ExternalInput")
    biaspk = nc.dram_tensor("biaspk", [128, 1024], F32, kind="ExternalInput")
    hT0 = nc.dram_tensor("hT0", [128, NK * BP], F32R, kind="ExternalInput")
    o0T = nc.dram_tensor("o0T", [128, BP], F32R, kind="ExternalInput")
    c0pk = nc.dram_tensor("c0pk", [128, 1024], F32, kind="ExternalInput")
    ident = nc.dram_tensor("ident", [32, 32], F32, kind="ExternalInput")

    HT = nc.dram_tensor("HT", [t_steps, 128, NK, BS], F32)  # internal staging
    yT = nc.dram_tensor("yT", [128, t_steps * BS], F32, kind="ExternalOutput")

    with tile.TileContext(nc) as tc:
        from contextlib import ExitStack

        with ExitStack() as perm:
            pw = perm.enter_context(tc.tile_pool(name="weights", bufs=1))
            Wstr_sb = pw.tile([128, NK * NCH * CW], F32R)
            WoT_sb = pw.tile([128, NK * O], F32R)
            nc.sync.dma_start(Wstr_sb[:], Wstr[:])
            nc.sync.dma_start(WoT_sb[:], WoT[:])

            with ExitStack() as rec:
                pr = rec.enter_context(tc.tile_pool(name="recur", bufs=1))
                Wx_sb = pr.tile([128, NCH * CW], F32R)
                Wio_sb = pr.tile([128, NCH * CW], F32R)
                bias_sb = pr.tile([128, 1024], F32)
                id_sb = pr.tile([32, 32], F32)
                x_sb = pr.tile([128, unroll * BP], F32R)
                x0_sb = pr.tile([128, BP], F32R)
                o0_sb = pr.tile([128, BP], F32R)
                hT_sb = pr.tile([128, NK * BP], F32R)
                c_sb = pr.tile([128, 1024], F32)     # rows 0:32 used
                tmp_sb = pr.tile([128, 1024], F32)   # tanh(g~), then i*g~
                h_sb = pr.tile([128, 1024], F32)     # tanh(c), then h

                pp = rec.enter_context(tc.tile_pool(name="gpsum", bufs=1, space="PSUM"))
                ps = pp.tile([128, NCH * CW], F32, name="gps")  # all 8 banks

                nc.sync.dma_start(Wx_sb[:], Wx[:])
                nc.sync.dma_start(Wio_sb[:], Wio[:])
                nc.sync.dma_start(bias_sb[:], biaspk[:])
                nc.sync.dma_start(id_sb[:], ident[:])
                nc.sync.dma_start(x0_sb[:], xT[:, 0:BP])
                nc.sync.dma_start(o0_sb[:], o0T[:])
                nc.sync.dma_start(hT_sb[:], hT0[:])
                nc.sync.dma_start(c_sb[0:BP, :], c0pk[0:BP, :])

                def emit_step(t_ap, x_stat, extra=False):
                    rounds = [(x_stat, Wx_sb)]
                    if extra:
                        rounds.append((o0_sb[:], Wio_sb))
                    for k in range(NK):
                        rounds.append((hT_sb[:, k * BP:(k + 1) * BP], None))
                    nr = len(rounds)
                    for n in range(NCH):
                        cs = slice(CW * n, CW * n + CW)
                        for ri, (stat, rhs_src) in enumerate(rounds):
                            if rhs_src is None:
                                k = ri - (nr - NK)
                                rhs = Wstr_sb[:, (k * NCH + n) * CW:(k * NCH + n + 1) * CW]
                            else:
                                rhs = rhs_src[:, cs]
                            nc.tensor.matmul(
                                ps[0:BP, cs], stat, rhs,
                                start=(ri == 0), stop=(ri == nr - 1),
                                skip_group_check=True,
                            )
                        # bias (packed [128,1024]: row block n//2, col half n%2)
                        q, hh = n // 2, n % 2
                        nc.vector.tensor_add(
                            ps[0:BP, cs], ps[0:BP, cs],
                            bias_sb[32 * q:32 * q + BP, 512 * hh:512 * hh + CW],
                        )
                        ctype = CHUNK_TYPE[n]
                        if ctype == 2:   # g~ -> tanh into SBUF (tmp)
                            nc.scalar.activation(
                                tmp_sb[0:BP, 512 * hh:512 * hh + CW],
                                ps[0:BP, cs], AF.Tanh)
                        else:            # i, f, o -> sigmoid in place
                            nc.scalar.activation(ps[0:BP, cs], ps[0:BP, cs],
                                                 AF.Sigmoid)
                        if n == 3:       # i (psum 0:1024) * tanh_g (tmp)
                            nc.vector.tensor_mul(
                                tmp_sb[0:BP, :], ps[0:BP, 0:1024], tmp_sb[0:BP, :])
                        if n == 5:       # c = c*f + tmp
                            nc.vector.tensor_mul(
                                c_sb[0:BP, :], c_sb[0:BP, :], ps[0:BP, 2048:3072])
                            nc.vector.tensor_add(
                                c_sb[0:BP, :], c_sb[0:BP, :], tmp_sb[0:BP, :])
                            nc.scalar.activation(h_sb[0:BP, :], c_sb[0:BP, :],
                                                 AF.Tanh)
                        if n == 7:       # h = o * tanh(c)
                            nc.vector.tensor_mul(
                                h_sb[0:BP, :], ps[0:BP, 3072:4096], h_sb[0:BP, :])

                    # transposes into psum bank0 (chunk 0 long consumed)
                    for k in range(NK):
                        nc.tensor.matmul(
                            ps[:, 32 * k:32 * k + 32],
                            h_sb[0:BP, 128 * k:128 * k + 128],
                            id_sb[:],
                            is_transpose=True,
                            skip_group_check=True,
                        )
                    nc.vector.tensor_copy(hT_sb[:], ps[:, 0:NK * BP])
                    src = hT_sb[:].bitcast(F32).rearrange(
                        "p (k b) -> p k b", b=BP)[:, :, 0:BS]
                    nc.sync.dma_start(HT[t_ap], src)

                emit_step(0, x0_sb[:], extra=True)

                with tc.For_i(0, n_iters, 1) as iv:
                    nc.sync.dma_start(
                        x_sb[:], xT[:, bass.ds((iv * unroll + 1) * BP, unroll * BP)]
                    )
                    for s in range(unroll):
                        emit_step(iv * unroll + (1 + s),
                                  x_sb[:, s * BP:(s + 1) * BP])

            # ---------------- final phase: y^T = W_out @ H^T ----------------
            with ExitStack() as fin:
                pf = fin.enter_context(tc.tile_pool(name="final", bufs=2))
                pfp = fin.enter_context(tc.tile_pool(name="ypsum", bufs=2, space="PSUM"))
                HT_r = HT[:].rearrange("t p k b -> p t k b")
                for blk in range((t_steps - 1) // final_tblk):
                    t0 = blk * final_tblk
                    hblk = pf.tile([128, final_tblk, NK, BS], F32R, name="hblk")
                    nc.sync.dma_start(hblk[:].bitcast(F32), HT_r[:, t0:t0 + final_tblk])
                    yps = pfp.tile([128, final_tblk * BS], F32, name="yps")
                    for k in range(NK):
                        nc.tensor.matmul(
                            yps[:],
                            WoT_sb[:, k * O:(k + 1) * O],
                            hblk[:, :, k, :],
                            start=(k == 0),
                            stop=(k == NK - 1),
                        )
                    y_sb = pf.tile([128, final_tblk * BS], F32, name="ysb")
                    nc.vector.tensor_copy(y_sb[:], yps[:])
                    nc.sync.dma_start(yT[:, t0 * BS:(t0 + final_tblk) * BS], y_sb[:])

    return nc


# ----------------------------------------------------------------------------
# Host-side packing
# ----------------------------------------------------------------------------

def make_core_inputs(xc, hnc, cnc, o0c_raw, W_ih, W_hh, b_ih, b_hh, W_out,
                     b_out, t_steps, shared):
    """Per-core tensors. xc [BS, T_in, I] already masked; T_in <= t_steps."""
    f32 = np.float32
    T_in = xc.shape[1]
    xTc = np.zeros((128, t_steps * BP), f32)
    xTc.reshape(128, t_steps, BP)[:, :T_in, :BS] = xc.transpose(2, 1, 0)
    hTc = np.zeros((128, NK * BP), f32)
    hTc.reshape(128, NK, BP)[:, :, :BS] = hnc.T.reshape(NK, 128, BS).transpose(1, 0, 2)
    o0_corr = o0c_raw - (hnc @ W_out.T + b_out)
    o0c = np.zeros((128, BP), f32)
    o0c[:, :BS] = o0_corr.T
    c0c = np.zeros((128, 1024), f32)
    c0c[:BS, :] = cnc
    out = {"xT": xTc, "hT0": hTc, "o0T": o0c, "c0pk": c0c}
    out.update(shared)
    return out


def pack_shared(W_ih, W_hh, b_ih, b_hh, W_out, b_out):
    f32 = np.float32
    W_ihx = W_ih[:, :I]
    W_io = W_ih[:, I:I + O]
    W_eff = (W_hh.astype(np.float64)
             + W_io.astype(np.float64) @ W_out.astype(np.float64)).astype(f32)
    b_eff = (b_ih.astype(np.float64) + b_hh.astype(np.float64)
             + W_io.astype(np.float64) @ b_out.astype(np.float64)).astype(f32)

    Wstr = np.empty((128, NK * NCH * CW), f32)
    WeffT = W_eff.T
    for k in range(NK):
        for n in range(NCH):
            r0, r1 = chunk_rows(n)
            Wstr[:, (k * NCH + n) * CW:(k * NCH + n + 1) * CW] = \
                WeffT[128 * k:128 * (k + 1), r0:r1]

    def pack_chunks(Wt):
        out = np.empty((128, NCH * CW), f32)
        for n in range(NCH):
            r0, r1 = chunk_rows(n)
            out[:, n * CW:(n + 1) * CW] = Wt[:, r0:r1]
        return out

    biaspk = np.empty((128, 1024), f32)
    for n in range(NCH):
        q, hh = n // 2, n % 2
        r0, r1 = chunk_rows(n)
        biaspk[32 * q:32 * (q + 1), 512 * hh:512 * hh + CW] = b_eff[r0:r1][None, :]

    WoT_p = np.empty((128, NK * O), f32)
    for k in range(NK):
        WoT_p[:, k * O:(k + 1) * O] = W_out[:, 128 * k:128 * (k + 1)].T

    return {
        "Wstr": Wstr, "Wx": pack_chunks(W_ihx.T), "Wio": pack_chunks(W_io.T),
        "WoT": WoT_p, "biaspk": biaspk,
        "ident": np.eye(32, dtype=f32),
    }


def _pack_inputs(x, sequence_length, hn, cn, output_t,
                 W_ih, W_hh, b_ih, b_hh, W_out, b_out, t_steps):
    f32 = np.float32
    x = np.asarray(x, f32)
    hn = np.asarray(hn, f32)
    cn = np.asarray(cn, f32)
    output_t = np.asarray(output_t, f32)
    W_ih = np.asarray(W_ih, f32)
    W_hh = np.asarray(W_hh, f32)
    b_ih = np.asarray(b_ih, f32)
    b_hh = np.asarray(b_hh, f32)
    W_out = np.asarray(W_out, f32)
    b_out = np.asarray(b_out, f32)
    seq = np.asarray(sequence_length).astype(np.int64)

    mask = (np.arange(T)[None, :] < seq[:, None])          # [B, T] bool
    xm = x * mask[:, :, None].astype(f32)

    shared = pack_shared(W_ih, W_hh, b_ih, b_hh, W_out, b_out)
    per_core = []
    for core in range(NCORES):
        bsl = slice(core * BS, (core + 1) * BS)
        per_core.append(make_core_inputs(
            xm[bsl], hn[bsl], cn[bsl], output_t[bsl],
            W_ih, W_hh, b_ih, b_hh, W_out, b_out, t_steps, shared))
    return per_core, mask, b_out


_CACHE = {}


def kernel(**inputs) -> np.ndarray:
    t_steps = T + 1  # one padded step so the loop is 1024 = 8*128
    key = ("nc", t_steps)
    if key not in _CACHE:
        nc_new = build_program(t_steps=t_steps)
        nc_new.compile()
        _CACHE[key] = nc_new
    nc = _CACHE[key]

    per_core, mask, b_out = _pack_inputs(t_steps=t_steps, **inputs)
    res = run_bass_kernel_spmd(nc, per_core, core_ids=list(range(NCORES)))

    y = np.empty((B, T, O), dtype=np.float32)
    for core in range(NCORES):
        yTc = res.results[core]["yT"]                       # [128, t_steps*BS]
        yc = yTc.reshape(128, t_steps, BS)[:, :T, :]        # [O, T, BS]
        y[core * BS:(core + 1) * BS] = yc.transpose(2, 1, 0)
    y += np.asarray(b_out, np.float32)[None, None, :]
    y *= mask[:, :, None].astype(np.float32)
    return y


# revision 18
# speedup vs baseline: 4.9889x; 4.9889x over previous
"""AutoRegressiveLSTM Trainium2 kernel (8-core data-parallel).

Per core: batch shard of 16 (padded to 32 on the stationary side).

  - Output feedback folded into the recurrence:
        out_{t-1} = h_{t-1} @ W_out.T + b_out
        gates_t   = x_t m_t @ W_ihx.T + h_{t-1} @ W_eff.T + b_eff
        W_eff = W_hh + W_io @ W_out,  b_eff = b_ih + b_hh + W_io @ b_out
    (t=0 adds one extra matmul round with a host-computed correction
     stationary o0 = output_t - (hn @ W_out.T + b_out))
  - All matmul operands are float16 (1 cycle/row on the PE); PSUM
    accumulation is fp32. Stationary = h^T [128 x 32] batch columns,
    moving = W_eff.T in 512-wide chunks. Gates fill PSUM [32 x 4096]
    (8 banks), chunk order i,i,g~,g~,f,f,o,o for early cell math.
  - Cell math: DVE adds bias in place in PSUM, ACT applies sigmoid/tanh
    in place (tanh g~ to SBUF), DVE combines; c stays fp32, h is f16.
  - h^T via 8 PE transpose ops (f16) into a bitcast view of PSUM bank0
    (consumed by then); split 4+4 around the o2 chunk to shorten the
    critical tail. h^T is stored contiguously to HBM (f16).
  - Final phase: y^T = W_out @ H^T as a batched matmul over all steps.
Host side does masking, sharding, packing, W_eff fold, final transpose.
"""

import numpy as np

import concourse.bass as bass
import concourse.mybir as mybir
import concourse.tile as tile
from concourse import bacc
from concourse.bass_utils import run_bass_kernel_spmd

F32 = mybir.dt.float32
F16 = mybir.dt.float16
AF = mybir.ActivationFunctionType
ET = mybir.EngineType

B, T, I, H, O = 128, 1024, 128, 1024, 128
NCORES = 8
BS = B // NCORES          # real batch per core (16)
BP = 32                   # padded batch per core
G = 4 * H                 # 4096 gates
NK = H // 128             # 8 h k-tiles
NCH = 8                   # gate chunks of 512
CW = 512                  # chunk width
# chunk n holds reference-gate rows 1024*CHUNK_TYPE[n] + 512*CHUNK_HALF[n] +:512
# reference gate order: i(0), f(1), g(2), o(3); we order i,i,g,g,f,f,o,o
CHUNK_TYPE = [0, 0, 2, 2, 1, 1, 3, 3]
CHUNK_HALF = [0, 1, 0, 1, 0, 1, 0, 1]


def chunk_rows(n):
    r0 = 1024 * CHUNK_TYPE[n] + 512 * CHUNK_HALF[n]
    return r0, r0 + CW


def build_program(t_steps=T + 1, unroll=8, final_tblk=32, ablate=()):
    """t_steps includes t=0; the loop runs (t_steps-1) steps (divisible by
    unroll). ablate: subset of {"store","transpose","elementwise"}."""
    assert (t_steps - 1) % unroll == 0
    n_iters = (t_steps - 1) // unroll

    nc = bacc.Bacc(
        "TRN2", target_bir_lowering=False, debug=False, num_devices=NCORES
    )

    # xT padded by one extra body for the in-loop prefetch
    xT = nc.dram_tensor("xT", [128, (t_steps + unroll) * BP], F16, kind="ExternalInput")
    Wstr = nc.dram_tensor("Wstr", [128, NK * NCH * CW], F16, kind="ExternalInput")
    Wx = nc.dram_tensor("Wx", [128, NCH * CW], F16, kind="ExternalInput")
    Wio = nc.dram_tensor("Wio", [128, NCH * CW], F16, kind="ExternalInput")
    WoT = nc.dram_tensor("WoT", [128, NK * O], F16, kind="ExternalInput")
    biaspk = nc.dram_tensor("biaspk", [128, 1024], F32, kind="ExternalInput")
    hT0 = nc.dram_tensor("hT0", [128, NK * BP], F16, kind="ExternalInput")
    o0T = nc.dram_tensor("o0T", [128, BP], F16, kind="ExternalInput")
    c0pk = nc.dram_tensor("c0pk", [128, 1024], F32, kind="ExternalInput")
    ident = nc.dram_tensor("ident", [32, 32], F16, kind="ExternalInput")

    HT = nc.dram_tensor("HT", [t_steps, 128, NK * BP], F16)  # internal staging
    yT = nc.dram_tensor("yT", [128, t_steps * BS], F32, kind="ExternalOutput")

    with tile.TileContext(nc) as tc:
        from contextlib import ExitStack

        with ExitStack() as perm:
            pw = perm.enter_context(tc.tile_pool(name="weights", bufs=1))
            Wstr_sb = pw.tile([128, NK * NCH * CW], F16)
            WoT_sb = pw.tile([128, NK * O], F16)
            nc.sync.dma_start(Wstr_sb[:], Wstr[:])
            nc.sync.dma_start(WoT_sb[:], WoT[:])

            with ExitStack() as rec:
                pr = rec.enter_context(tc.tile_pool(name="recur", bufs=1))
                Wx_sb = pr.tile([128, NCH * CW], F16)
                Wio_sb = pr.tile([128, NCH * CW], F16)
                bias_sb = pr.tile([128, 1024], F32)
                id_sb = pr.tile([32, 32], F16)
                x_sb = pr.tile([128, unroll * BP], F16)
                x0_sb = pr.tile([128, BP], F16)
                o0_sb = pr.tile([128, BP], F16)
                hT_sb = pr.tile([128, NK * BP], F16)
                c_sb = pr.tile([128, 1024], F32)     # rows 0:32 used
                tmp_sb = pr.tile([128, 1024], F32)   # tanh(g~), then i*g~
                h_sb = pr.tile([128, 1024], F16)     # tanh(c), then h

                pp = rec.enter_context(tc.tile_pool(name="gpsum", bufs=1, space="PSUM"))
                ps = pp.tile([128, NCH * CW], F32, name="gps")  # all 8 banks
                # f16 view of bank0's first half, for transpose outputs
                ps16 = ps[:, 0:NK * BP // 2].bitcast(F16)       # [128, NK*BP]

                nc.sync.dma_start(Wx_sb[:], Wx[:])
                nc.sync.dma_start(Wio_sb[:], Wio[:])
                nc.sync.dma_start(bias_sb[:], biaspk[:])
                nc.sync.dma_start(id_sb[:], ident[:])
                nc.sync.dma_start(x0_sb[:], xT[:, 0:BP])
                nc.sync.dma_start(o0_sb[:], o0T[:])
                nc.sync.dma_start(hT_sb[:], hT0[:])
                nc.sync.dma_start(c_sb[0:BP, :], c0pk[0:BP, :])
                # prefetch body 0's x
                nc.sync.dma_start(x_sb[:], xT[:, BP:(unroll + 1) * BP])

                def transpose_half(first):
                    ks = range(0, 4) if first else range(4, NK)
                    for k in ks:
                        nc.tensor.matmul(
                            ps16[:, 32 * k:32 * k + 32],
                            h_sb[0:BP, 128 * k:128 * k + 128],
                            id_sb[:],
                            is_transpose=True,
                            skip_group_check=True,
                        )

                def emit_step(t_ap, x_stat, extra=False):
                    rounds = [(x_stat, Wx_sb)]
                    if extra:
                        rounds.append((o0_sb[:], Wio_sb))
                    for k in range(NK):
                        rounds.append((hT_sb[:, k * BP:(k + 1) * BP], None))
                    nr = len(rounds)
                    for n in range(NCH):
                        cs = slice(CW * n, CW * n + CW)
                        for ri, (stat, rhs_src) in enumerate(rounds):
                            if rhs_src is None:
                                k = ri - (nr - NK)
                                rhs = Wstr_sb[:, (k * NCH + n) * CW:(k * NCH + n + 1) * CW]
                            else:
                                rhs = rhs_src[:, cs]
                            nc.tensor.matmul(
                                ps[0:BP, cs], stat, rhs,
                                start=(ri == 0), stop=(ri == nr - 1),
                                skip_group_check=True,
                            )
                        if n == 7 and "transpose" not in ablate and \
                                "elementwise" not in ablate:
                            # h[0:512] ready while chunk 7 streams
                            transpose_half(True)
                        if "elementwise" in ablate:
                            continue
                        # ---- cell math (pairs merged where slack allows) ----
                        q, hh = n // 2, n % 2
                        pair = slice(CW * (n - 1), CW * (n + 1))
                        if n in (1, 3, 5):     # merged ops for i, g~, f pairs
                            nc.vector.tensor_add(
                                ps[0:BP, pair], ps[0:BP, pair],
                                bias_sb[32 * q:32 * q + BP, :])
                            if n == 3:         # tanh(g~) -> tmp
                                nc.scalar.activation(
                                    tmp_sb[0:BP, :], ps[0:BP, pair], AF.Tanh)
                                # i in place, then tmp = sig_i * tanh_g
                                nc.scalar.activation(
                                    ps[0:BP, 0:1024], ps[0:BP, 0:1024], AF.Sigmoid)
                                nc.vector.tensor_mul(
                                    tmp_sb[0:BP, :], ps[0:BP, 0:1024], tmp_sb[0:BP, :])
                            elif n == 1:
                                pass           # sigmoid folded into n==3 above
                            elif n == 5:       # f: c = sig(f)*c + tmp
                                nc.scalar.activation(
                                    ps[0:BP, pair], ps[0:BP, pair], AF.Sigmoid)
                                nc.vector.tensor_mul(
                                    c_sb[0:BP, :], c_sb[0:BP, :], ps[0:BP, pair])
                                nc.vector.tensor_add(
                                    c_sb[0:BP, :], c_sb[0:BP, :], tmp_sb[0:BP, :])
                                nc.scalar.activation(h_sb[0:BP, :], c_sb[0:BP, :],
                                                     AF.Tanh)
                        elif n in (6, 7):      # o chunks kept split (tail)
                            nc.vector.tensor_add(
                                ps[0:BP, cs], ps[0:BP, cs],
                                bias_sb[96:96 + BP, 512 * hh:512 * hh + CW])
                            nc.scalar.activation(ps[0:BP, cs], ps[0:BP, cs],
                                                 AF.Sigmoid)
                            h_half = slice(512 * hh, 512 * hh + CW)
                            nc.vector.tensor_mul(
                                h_sb[0:BP, h_half], ps[0:BP, cs],
                                h_sb[0:BP, h_half])

                    if "transpose" in ablate or "elementwise" in ablate:
                        return
                    transpose_half(False)
                    nc.vector.tensor_copy(hT_sb[:], ps16[:, 0:NK * BP])
                    if "store" in ablate:
                        return
                    nc.sync.dma_start(HT[t_ap], hT_sb[:])

                emit_step(0, x0_sb[:], extra=True)

                with tc.For_i(0, n_iters, 1, hint_engines=(ET.PE,)) as iv:
                    for s in range(unroll):
                        emit_step(iv * unroll + (1 + s),
                                  x_sb[:, s * BP:(s + 1) * BP])
                    # prefetch next body's x (xT is padded by one body)
                    nc.sync.dma_start(
                        x_sb[:],
                        xT[:, bass.ds(((iv + 1) * unroll + 1) * BP, unroll * BP)]
                    )

            # ---------------- final phase: y^T = W_out @ H^T ----------------
            with ExitStack() as fin:
                pf = fin.enter_context(tc.tile_pool(name="final", bufs=2))
                pfp = fin.enter_context(tc.tile_pool(name="ypsum", bufs=2, space="PSUM"))
                # HT [t, p, k*32+b] -> [p, t, k, b]
                HT_r = HT[:].rearrange("t p (k b) -> p t k b", b=BP)
                for blk in range((t_steps - 1) // final_tblk):
                    t0 = blk * final_tblk
                    hblk = pf.tile([128, final_tblk, NK, BP], F16, name="hblk")
                    nc.sync.dma_start(hblk[:], HT_r[:, t0:t0 + final_tblk])
                    yps = pfp.tile([128, final_tblk * BS], F32, name="yps")
                    for k in range(NK):
                        nc.tensor.matmul(
                            yps[:],
                            WoT_sb[:, k * O:(k + 1) * O],
                            hblk[:, :, k, 0:BS],
                            start=(k == 0),
                            stop=(k == NK - 1),
                        )
                    y_sb = pf.tile([128, final_tblk * BS], F32, name="ysb")
                    nc.vector.tensor_copy(y_sb[:], yps[:])
                    nc.sync.dma_start(yT[:, t0 * BS:(t0 + final_tblk) * BS], y_sb[:])

    return nc


# ----------------------------------------------------------------------------
# Host-side packing
# ----------------------------------------------------------------------------

def make_core_inputs(xc, hnc, cnc, o0c_raw, W_ih, W_hh, b_ih, b_hh, W_out,
                     b_out, t_steps, shared, unroll=8):
    """Per-core tensors. xc [BS, T_in, I] already masked; T_in <= t_steps."""
    f16 = np.float16
    T_in = xc.shape[1]
    xTc = np.zeros((128, (t_steps + unroll) * BP), f16)
    xTc.reshape(128, t_steps + unroll, BP)[:, :T_in, :BS] = \
        xc.transpose(2, 1, 0).astype(f16)
    hTc = np.zeros((128, NK * BP), f16)
    hTc.reshape(128, NK, BP)[:, :, :BS] = \
        hnc.T.reshape(NK, 128, BS).transpose(1, 0, 2).astype(f16)
    o0_corr = o0c_raw - (hnc @ W_out.T + b_out)
    o0c = np.zeros((128, BP), f16)
    o0c[:, :BS] = o0_corr.T.astype(f16)
    c0c = np.zeros((128, 1024), np.float32)
    c0c[:BS, :] = cnc
    out = {"xT": xTc, "hT0": hTc, "o0T": o0c, "c0pk": c0c}
    out.update(shared)
    return out


def pack_shared(W_ih, W_hh, b_ih, b_hh, W_out, b_out):
    f32, f16 = np.float32, np.float16
    W_ihx = W_ih[:, :I]
    W_io = W_ih[:, I:I + O]
    W_eff = (W_hh.astype(np.float64)
             + W_io.astype(np.float64) @ W_out.astype(np.float64)).astype(f32)
    b_eff = (b_ih.astype(np.float64) + b_hh.astype(np.float64)
             + W_io.astype(np.float64) @ b_out.astype(np.float64)).astype(f32)

    Wstr = np.empty((128, NK * NCH * CW), f16)
    WeffT = W_eff.T
    for k in range(NK):
        for n in range(NCH):
            r0, r1 = chunk_rows(n)
            Wstr[:, (k * NCH + n) * CW:(k * NCH + n + 1) * CW] = \
                WeffT[128 * k:128 * (k + 1), r0:r1].astype(f16)

    def pack_chunks(Wt):
        out = np.empty((128, NCH * CW), f16)
        for n in range(NCH):
            r0, r1 = chunk_rows(n)
            out[:, n * CW:(n + 1) * CW] = Wt[:, r0:r1].astype(f16)
        return out

    biaspk = np.empty((128, 1024), f32)
    for n in range(NCH):
        q, hh = n // 2, n % 2
        r0, r1 = chunk_rows(n)
        biaspk[32 * q:32 * (q + 1), 512 * hh:512 * hh + CW] = b_eff[r0:r1][None, :]

    WoT_p = np.empty((128, NK * O), f16)
    for k in range(NK):
        WoT_p[:, k * O:(k + 1) * O] = W_out[:, 128 * k:128 * (k + 1)].T.astype(f16)

    return {
        "Wstr": Wstr, "Wx": pack_chunks(W_ihx.T), "Wio": pack_chunks(W_io.T),
        "WoT": WoT_p, "biaspk": biaspk,
        "ident": np.eye(32, dtype=f16),
    }


def _pack_inputs(x, sequence_length, hn, cn, output_t,
                 W_ih, W_hh, b_ih, b_hh, W_out, b_out, t_steps):
    f32 = np.float32
    x = np.asarray(x, f32)
    hn = np.asarray(hn, f32)
    cn = np.asarray(cn, f32)
    output_t = np.asarray(output_t, f32)
    W_ih = np.asarray(W_ih, f32)
    W_hh = np.asarray(W_hh, f32)
    b_ih = np.asarray(b_ih, f32)
    b_hh = np.asarray(b_hh, f32)
    W_out = np.asarray(W_out, f32)
    b_out = np.asarray(b_out, f32)
    seq = np.asarray(sequence_length).astype(np.int64)

    mask = (np.arange(T)[None, :] < seq[:, None])          # [B, T] bool
    xm = x * mask[:, :, None].astype(f32)

    shared = pack_shared(W_ih, W_hh, b_ih, b_hh, W_out, b_out)
    per_core = []
    for core in range(NCORES):
        bsl = slice(core * BS, (core + 1) * BS)
        per_core.append(make_core_inputs(
            xm[bsl], hn[bsl], cn[bsl], output_t[bsl],
            W_ih, W_hh, b_ih, b_hh, W_out, b_out, t_steps, shared))
    return per_core, mask, b_out


_CACHE = {}


def kernel(**inputs) -> np.ndarray:
    t_steps = T + 1  # one padded step so the loop is 1024 = 8*128
    key = ("nc", t_steps)
    if key not in _CACHE:
        nc_new = build_program(t_steps=t_steps)
        nc_new.compile()
        _CACHE[key] = nc_new
    nc = _CACHE[key]

    per_core, mask, b_out = _pack_inputs(t_steps=t_steps, **inputs)
    res = run_bass_kernel_spmd(nc, per_core, core_ids=list(range(NCORES)))

    y = np.empty((B, T, O), dtype=np.float32)
    for core in range(NCORES):
        yTc = res.results[core]["yT"]                       # [128, t_steps*BS]
        yc = yTc.reshape(128, t_steps, BS)[:, :T, :]        # [O, T, BS]
        y[core * BS:(core + 1) * BS] = yc.transpose(2, 1, 0)
    y += np.asarray(b_out, np.float32)[None, None, :]
    y *= mask[:, :, None].astype(np.float32)
    return y
